# revision 11
# baseline (speedup 1.0000x reference)
"""Sparse masked attention on 8 TRN2 NeuronCores.

reference:  O = softmax((Q K^T * mq[:,None] + log(mk[None,:])) / 8) @ V
  - rows with mq=0: scores all equal -> uniform average of V over mk=1 keys
  - keys with mk=0: exactly dropped from the softmax

Strategy: batch (b=8) is data-parallel across the 8 cores. On the host we
compact each batch to its unmasked queries/keys (~n/2 each), so each core
computes a dense ~2100x2176 attention instead of 4096x4096:

  S^T[chunk, w] = (Kc^T chunk).T @ Qc^T      (TensorE bf16, d=64, row-group
                                              pairs into 2-bank PSUM tiles)
  P^T = exp(S^T / 8) -> bf16                 (pair-wide [128,2,w] instrs;
                                              ScalarE exact exp / VectorE
                                              Schraudolph fast-exp,
                                              ns-balanced in runs of 2)
  O^T[65, w] += Vext[chunk].T @ P^T          (TensorE bf16, K=128 full
                                              height, single accumulator)

where Vext = [V | 1]: the ones column accumulates the softmax denominator.
No row-max subtraction is needed: scores/8 ~ N(0,1), exp stays in range.
The host divides by the denominator, scatters rows back, and fills masked
query rows with mean(V[mk=1]).
"""

import numpy as np
import ml_dtypes

N_CORES = 8
W = 512   # max n-block width (PSUM bank / fp32 matmul free-dim limit)

# exp engine cost model (ns) for ACT/DVE balancing
_ACT_COL = 0.833
_DVE_COL = 1.042
_ACT_FIX = 255.0
_DVE_FIX = 195.0

# Schraudolph fast exp on bf16 bit pattern, via int16 (DVE rounds):
#   i16 = round(s * (2^7/ln2)/8 + (127*2^7 - C)) ; bitcast -> bf16
_A16 = float(2 ** 7 / np.log(2.0) / 8.0)
_B16 = float(127 * 2 ** 7 - 7.5)


def _round_up(x, mult):
    return ((x + mult - 1) // mult) * mult


def _blocks_of(ncap):
    blocks, off = [], 0
    while off < ncap:
        w = min(W, ncap - off)
        blocks.append((off, w))
        off += w
    return blocks


def _slot_engines(blocks, npairs):
    """Engine per (block, pair) slot, emitted in that order. Greedy ns
    balance with preferred run length 2."""
    out = []
    la = ld = 0.0
    run_eng, run_len = None, 0
    for (j0, w) in blocks:
        for p in range(npairs):
            ca = 2 * w * _ACT_COL + _ACT_FIX
            cd = 2 * w * _DVE_COL + _DVE_FIX
            if run_eng is not None and run_len < 2:
                eng = run_eng
            else:
                eng = "act" if la + ca <= ld + cd else "dve"
            if eng == run_eng:
                run_len += 1
            else:
                run_eng, run_len = eng, 1
            if eng == "act":
                la += ca
            else:
                ld += cd
            out.append(eng)
    return out, la, ld


_build_cache = {}


def _build(ncap, mcap):
    """Per-core graph. Inputs (per core):
      qt   [128, ncap]          bf16  two stacked copies of Q^T (compacted)
      ktp  [128, npairs*128]    bf16  K^T chunk pairs: pair p = chunk 2p on
                                      partitions 0-63, chunk 2p+1 on 64-127
      vext [128, mchunks*65]    bf16  partition-major Vext chunks: partition
                                      r, cols [c*65:(c+1)*65] = Vext row
                                      c*128+r = [V row | 1.0] (0 if padding)
    Output: out [nblocks*65, W] f32: block j rows [65j:65j+65], cols [:w_j] =
      [ O^T numerator (64 rows) ; denominator (1 row) ] for n-cols j*W..+w_j.
    """
    key = (ncap, mcap)
    if key in _build_cache:
        return _build_cache[key]

    import concourse.bacc as bacc
    import concourse.mybir as mybir
    import concourse.tile as tile

    f32 = mybir.dt.float32
    bf16 = mybir.dt.bfloat16
    i16 = mybir.dt.int16
    EXP = mybir.ActivationFunctionType.Exp

    mchunks = mcap // 128
    npairs = (mchunks + 1) // 2
    blocks = _blocks_of(ncap)
    nblocks = len(blocks)
    engs, la, ld = _slot_engines(blocks, npairs)

    nc = bacc.Bacc("TRN2", target_bir_lowering=False, debug=False,
                   num_devices=N_CORES)
    qt_d = nc.dram_tensor("qt", [128, ncap], bf16, kind="ExternalInput")
    ktp_d = nc.dram_tensor("ktp", [128, npairs * 128], bf16,
                           kind="ExternalInput")
    vext_d = nc.dram_tensor("vext", [128, mchunks * 65], bf16,
                            kind="ExternalInput")
    out_d = nc.dram_tensor("out", [nblocks * 65, W], f32,
                           kind="ExternalOutput")

    load = {"act": la, "dve": ld}

    with tile.TileContext(nc) as tc:
        with (
            tc.tile_pool(name="resident", bufs=1) as resident,
            tc.tile_pool(name="pt", bufs=6) as ptp,
            tc.tile_pool(name="osb", bufs=2) as osbp,
            tc.tile_pool(name="psum_s", bufs=3, space="PSUM") as psum_s,
            tc.tile_pool(name="psum_o", bufs=2, space="PSUM") as psum_o,
        ):
            kt_sb = resident.tile([128, npairs * 128], bf16)
            qt_sb = resident.tile([128, ncap], bf16)
            v_sb = resident.tile([128, mchunks * 65], bf16)
            w0 = blocks[0][1]
            nc.sync.dma_start(kt_sb[:, 0:128], ktp_d[:, 0:128])
            nc.scalar.dma_start(qt_sb[:, 0:w0], qt_d[:, 0:w0])
            nc.sync.dma_start(v_sb[:, 0:130], vext_d[:, 0:130])
            nc.scalar.dma_start(qt_sb[:, w0:], qt_d[:, w0:])
            nc.sync.dma_start(kt_sb[:, 128:], ktp_d[:, 128:])
            nc.scalar.dma_start(v_sb[:, 130:], vext_d[:, 130:])

            def drain(jb, o, w, dma_eng):
                osb = osbp.tile([65, W], f32, tag="osb")
                ca = w * _ACT_COL + _ACT_FIX
                cd = w * _DVE_COL + _DVE_FIX
                if load["act"] + ca <= load["dve"] + cd:
                    load["act"] += ca
                    nc.scalar.copy(osb[:, 0:w], o[0:65, 0:w])
                else:
                    load["dve"] += cd
                    nc.vector.tensor_copy(osb[:, 0:w], o[0:65, 0:w])
                dma_eng.dma_start(out_d[jb * 65:(jb + 1) * 65, 0:w],
                                  osb[:, 0:w])

            pending_drain = None
            pend_pv = []  # [(p, o, pt2, w)] awaiting PV, lagged 2 slots
            slot = 0

            def pv(p, o, pt2, w, last):
                for c in range(2):
                    mi = 2 * p + c
                    if mi >= mchunks:
                        continue  # phantom chunk: V is zero, contributes 0
                    nc.tensor.matmul(
                        o[:, 0:w], v_sb[:, mi * 65:(mi + 1) * 65],
                        pt2[:, c, 0:w].bitcast(bf16),
                        start=(mi == 0), stop=last and mi == mchunks - 1,
                        tile_position=(0, 0), skip_group_check=True)

            for jb, (j0, w) in enumerate(blocks):
                o = psum_o.tile([65, W], f32, tag="o")
                for p in range(npairs):
                    st2 = psum_s.tile([128, 2, W], f32, tag="st2")
                    nc.tensor.matmul(
                        st2[:, 0, 0:w], kt_sb[0:64, p * 128:(p + 1) * 128],
                        qt_sb[0:64, j0:j0 + w],
                        start=True, stop=True, tile_position=(0, 0))
                    nc.tensor.matmul(
                        st2[:, 1, 0:w], kt_sb[64:128, p * 128:(p + 1) * 128],
                        qt_sb[64:128, j0:j0 + w],
                        start=True, stop=True, tile_position=(64, 0))
                    # PV lagged TWO slots: its exp finished ~2 exp-latencies
                    # ago, so the wait Tile moves onto the PV's LDWEIGHTS is
                    # already satisfied and the PE never stalls on exp
                    if len(pend_pv) >= 2:
                        q = pend_pv.pop(0)
                        pv(*q, last=(q[0] == npairs - 1))
                    eng = engs[slot]
                    pt2 = ptp.tile([128, 2, W], i16, tag="pt2")
                    if slot == 0:
                        # pipeline fill: split the first exp per-chunk across
                        # BOTH engines so PV(0) unblocks ~0.5us earlier
                        nc.scalar.activation(
                            pt2[:, 0, 0:w].bitcast(bf16),
                            st2[:, 0, 0:w], EXP, scale=0.125)
                        nc.vector.tensor_scalar(
                            pt2[:, 1, 0:w], st2[:, 1, 0:w], _A16, _B16,
                            mybir.AluOpType.mult, mybir.AluOpType.add)
                    elif eng == "act":
                        nc.scalar.activation(
                            pt2[:, :, 0:w].bitcast(bf16),
                            st2[:, :, 0:w], EXP, scale=0.125)
                    else:
                        nc.vector.tensor_scalar(
                            pt2[:, :, 0:w], st2[:, :, 0:w], _A16, _B16,
                            mybir.AluOpType.mult, mybir.AluOpType.add)
                    slot += 1
                    pend_pv.append((p, o, pt2, w))
                    if p == 1 and pending_drain is not None:
                        drain(*pending_drain)
                        pending_drain = None
                dma_eng = nc.sync if jb % 2 == 0 else nc.scalar
                pending_drain = (jb, o, w, dma_eng)
            for q in pend_pv:
                pv(*q, last=(q[0] == npairs - 1))
            drain(*pending_drain)

    nc.compile()
    _build_cache[key] = nc
    return nc


def _run(inputs, trace=False):
    queries = np.asarray(inputs["queries"], dtype=np.float32)
    keys = np.asarray(inputs["keys"], dtype=np.float32)
    values = np.asarray(inputs["values"], dtype=np.float32)
    mask_query = np.asarray(inputs["mask_query"])
    mask_key = np.asarray(inputs["mask_key"])

    b, n, d = queries.shape
    dv = values.shape[2]
    assert b == N_CORES, f"batch {b} != {N_CORES} cores"
    bf = ml_dtypes.bfloat16

    idx_q = [np.flatnonzero(mask_query[i]) for i in range(b)]
    idx_k = [np.flatnonzero(mask_key[i]) for i in range(b)]
    ncap = max(max(len(ix) for ix in idx_q), 64)
    mcap = _round_up(max(max(len(ix) for ix in idx_k), 1), 128)
    mchunks = mcap // 128
    npairs = (mchunks + 1) // 2
    blocks = _blocks_of(ncap)
    nblocks = len(blocks)

    qt = np.zeros((b, 128, ncap), bf)
    ktp = np.zeros((b, 128, npairs * 128), bf)
    vext = np.zeros((b, 128, mchunks * 65), bf)
    for i in range(b):
        nq, nk = len(idx_q[i]), len(idx_k[i])
        qc_t = queries[i, idx_q[i]].T.astype(bf)
        qt[i, 0:64, :nq] = qc_t
        qt[i, 64:128, :nq] = qc_t
        kc_t = np.zeros((64, mcap), np.float32)
        kc_t[:, :nk] = keys[i, idx_k[i]].T
        kc_t = kc_t.astype(bf)
        for p in range(npairs):
            ktp[i, 0:64, p * 128:(p + 1) * 128] = \
                kc_t[:, (2 * p) * 128:(2 * p + 1) * 128]
            if 2 * p + 1 < mchunks:
                ktp[i, 64:128, p * 128:(p + 1) * 128] = \
                    kc_t[:, (2 * p + 1) * 128:(2 * p + 2) * 128]
        ve = np.zeros((mcap, 65), np.float32)
        ve[:nk, :dv] = values[i, idx_k[i]]
        ve[:nk, dv] = 1.0
        vext[i] = ve.reshape(mchunks, 128, 65).transpose(1, 0, 2) \
                    .reshape(128, mchunks * 65).astype(bf)

    nc = _build(ncap, mcap)

    from concourse.bass_utils import run_bass_kernel_spmd
    in_maps = [{"qt": qt[i], "ktp": ktp[i], "vext": vext[i]}
               for i in range(b)]
    res = run_bass_kernel_spmd(nc, in_maps, core_ids=list(range(N_CORES)),
                               trace=trace)

    out = np.empty((b, n, dv), np.float32)
    for i in range(b):
        ot = res.results[i]["out"]
        nq, nk = len(idx_q[i]), len(idx_k[i])
        full = np.concatenate(
            [ot[jb * 65:(jb + 1) * 65, :w] for jb, (j0, w)
             in enumerate(blocks)], axis=1)
        num = full[:dv, :nq]
        den = full[dv, :nq]
        if nk > 0:
            out[i, :, :] = values[i, idx_k[i]].mean(axis=0)
        else:
            out[i, :, :] = 0.0
        if nq > 0:
            out[i, idx_q[i], :] = (num / den).T
    return out, res


def kernel(**inputs):
    out, _ = _run(inputs, trace=False)
    return out


# revision 12
# speedup vs baseline: 1.0227x; 1.0227x over previous
"""Sparse masked attention on 8 TRN2 NeuronCores.

reference:  O = softmax((Q K^T * mq[:,None] + log(mk[None,:])) / 8) @ V
  - rows with mq=0: scores all equal -> uniform average of V over mk=1 keys
  - keys with mk=0: exactly dropped from the softmax

Strategy: batch (b=8) is data-parallel across the 8 cores. On the host we
compact each batch to its unmasked queries/keys (~n/2 each), so each core
computes a dense ~2100x2176 attention instead of 4096x4096:

  S^T[chunk, w] = (Kc^T chunk).T @ Qc^T      (TensorE bf16, d=64, row-group
                                              pairs into 2-bank PSUM tiles)
  P^T = exp(S^T / 8) -> bf16                 (pair-wide [128,2,w] instrs;
                                              ScalarE exact exp / VectorE
                                              Schraudolph fast-exp,
                                              ns-balanced in runs of 2)
  O^T[65, w] += Vext[chunk].T @ P^T          (TensorE bf16, K=128 full
                                              height, single accumulator)

where Vext = [V | 1]: the ones column accumulates the softmax denominator.
No row-max subtraction is needed: scores/8 ~ N(0,1), exp stays in range.
The host divides by the denominator, scatters rows back, and fills masked
query rows with mean(V[mk=1]).
"""

import numpy as np
import ml_dtypes

N_CORES = 8
W = 512   # max n-block width (PSUM bank / fp32 matmul free-dim limit)

# exp engine cost model (ns) for ACT/DVE balancing
_ACT_COL = 0.833
_DVE_COL = 1.042
_ACT_FIX = 255.0
_DVE_FIX = 195.0

# Schraudolph fast exp on bf16 bit pattern, via int16 (DVE rounds):
#   i16 = round(s * (2^7/ln2)/8 + (127*2^7 - C)) ; bitcast -> bf16
_A16 = float(2 ** 7 / np.log(2.0) / 8.0)
_B16 = float(127 * 2 ** 7 - 7.5)


def _round_up(x, mult):
    return ((x + mult - 1) // mult) * mult


def _blocks_of(ncap):
    blocks, off = [], 0
    while off < ncap:
        w = min(W, ncap - off)
        blocks.append((off, w))
        off += w
    return blocks


def _slot_engines(blocks, npairs):
    """Engine per (block, pair) slot, emitted in that order. Greedy ns
    balance with preferred run length 2."""
    out = []
    la = ld = 0.0
    run_eng, run_len = None, 0
    for (j0, w) in blocks:
        for p in range(npairs):
            ca = 2 * w * _ACT_COL + _ACT_FIX
            cd = 2 * w * _DVE_COL + _DVE_FIX
            if run_eng is not None and run_len < 2:
                eng = run_eng
            else:
                eng = "act" if la + ca <= ld + cd else "dve"
            if eng == run_eng:
                run_len += 1
            else:
                run_eng, run_len = eng, 1
            if eng == "act":
                la += ca
            else:
                ld += cd
            out.append(eng)
    return out, la, ld


_build_cache = {}


def _build(ncap, mcap):
    """Per-core graph. Inputs (per core):
      qt   [128, ncap]          bf16  two stacked copies of Q^T (compacted)
      ktp  [128, npairs*128]    bf16  K^T chunk pairs: pair p = chunk 2p on
                                      partitions 0-63, chunk 2p+1 on 64-127
      vext [128, mchunks*65]    bf16  partition-major Vext chunks: partition
                                      r, cols [c*65:(c+1)*65] = Vext row
                                      c*128+r = [V row | 1.0] (0 if padding)
    Output: out [nblocks*65, W] f32: block j rows [65j:65j+65], cols [:w_j] =
      [ O^T numerator (64 rows) ; denominator (1 row) ] for n-cols j*W..+w_j.
    """
    key = (ncap, mcap)
    if key in _build_cache:
        return _build_cache[key]

    import concourse.bacc as bacc
    import concourse.mybir as mybir
    import concourse.tile as tile

    f32 = mybir.dt.float32
    bf16 = mybir.dt.bfloat16
    i16 = mybir.dt.int16
    EXP = mybir.ActivationFunctionType.Exp

    mchunks = mcap // 128
    npairs = (mchunks + 1) // 2
    blocks = _blocks_of(ncap)
    nblocks = len(blocks)
    engs, la, ld = _slot_engines(blocks, npairs)

    nc = bacc.Bacc("TRN2", target_bir_lowering=False, debug=False,
                   num_devices=N_CORES)
    qt_d = nc.dram_tensor("qt", [128, ncap], bf16, kind="ExternalInput")
    ktp_d = nc.dram_tensor("ktp", [128, npairs * 128], bf16,
                           kind="ExternalInput")
    vext_d = nc.dram_tensor("vext", [128, mchunks * 65], bf16,
                            kind="ExternalInput")
    out_d = nc.dram_tensor("out", [nblocks * 65, W], f32,
                           kind="ExternalOutput")

    load = {"act": la, "dve": ld}

    with tile.TileContext(nc) as tc:
        with (
            tc.tile_pool(name="resident", bufs=1) as resident,
            tc.tile_pool(name="pt", bufs=6) as ptp,
            tc.tile_pool(name="osb", bufs=2) as osbp,
            tc.tile_pool(name="psum_s", bufs=3, space="PSUM") as psum_s,
            tc.tile_pool(name="psum_o", bufs=2, space="PSUM") as psum_o,
        ):
            kt_sb = resident.tile([128, npairs * 128], bf16)
            qt_sb = resident.tile([128, ncap], bf16)
            v_sb = resident.tile([128, mchunks * 65], bf16)
            w0 = blocks[0][1]
            nc.sync.dma_start(kt_sb[:, 0:128], ktp_d[:, 0:128])
            nc.scalar.dma_start(qt_sb[:, 0:w0], qt_d[:, 0:w0])
            nc.sync.dma_start(v_sb[:, 0:130], vext_d[:, 0:130])
            nc.scalar.dma_start(qt_sb[:, w0:], qt_d[:, w0:])
            nc.sync.dma_start(kt_sb[:, 128:], ktp_d[:, 128:])
            nc.scalar.dma_start(v_sb[:, 130:], vext_d[:, 130:])

            def drain(jb, o, w, dma_eng):
                osb = osbp.tile([65, W], f32, tag="osb")
                ca = w * _ACT_COL + _ACT_FIX
                cd = w * _DVE_COL + _DVE_FIX
                if load["act"] + ca <= load["dve"] + cd:
                    load["act"] += ca
                    nc.scalar.copy(osb[:, 0:w], o[0:65, 0:w])
                else:
                    load["dve"] += cd
                    nc.vector.tensor_copy(osb[:, 0:w], o[0:65, 0:w])
                dma_eng.dma_start(out_d[jb * 65:(jb + 1) * 65, 0:w],
                                  osb[:, 0:w])

            pending_drain = None
            pend_pv = []  # [(p, o, pt2, w)] awaiting PV, lagged 2 slots
            slot = 0

            def pv(p, o, pt2, w, last):
                for c in range(2):
                    mi = 2 * p + c
                    if mi >= mchunks:
                        continue  # phantom chunk: V is zero, contributes 0
                    nc.tensor.matmul(
                        o[:, 0:w], v_sb[:, mi * 65:(mi + 1) * 65],
                        pt2[:, c, 0:w].bitcast(bf16),
                        start=(mi == 0), stop=last and mi == mchunks - 1,
                        tile_position=(0, 0), skip_group_check=True)

            for jb, (j0, w) in enumerate(blocks):
                o = psum_o.tile([65, W], f32, tag="o")
                for p in range(npairs):
                    st2 = psum_s.tile([128, 2, W], f32, tag="st2")
                    nc.tensor.matmul(
                        st2[:, 0, 0:w], kt_sb[0:64, p * 128:(p + 1) * 128],
                        qt_sb[0:64, j0:j0 + w],
                        start=True, stop=True, tile_position=(0, 0))
                    nc.tensor.matmul(
                        st2[:, 1, 0:w], kt_sb[64:128, p * 128:(p + 1) * 128],
                        qt_sb[64:128, j0:j0 + w],
                        start=True, stop=True, tile_position=(64, 0))
                    # PV lagged TWO slots: its exp finished ~2 exp-latencies
                    # ago, so the wait Tile moves onto the PV's LDWEIGHTS is
                    # already satisfied and the PE never stalls on exp
                    if len(pend_pv) >= 2:
                        q = pend_pv.pop(0)
                        pv(*q, last=(q[0] == npairs - 1))
                    eng = engs[slot]
                    slot += 1
                    pt2 = ptp.tile([128, 2, W], i16, tag="pt2")
                    if eng == "act":
                        nc.scalar.activation(
                            pt2[:, :, 0:w].bitcast(bf16),
                            st2[:, :, 0:w], EXP, scale=0.125)
                    else:
                        nc.vector.tensor_scalar(
                            pt2[:, :, 0:w], st2[:, :, 0:w], _A16, _B16,
                            mybir.AluOpType.mult, mybir.AluOpType.add)
                    pend_pv.append((p, o, pt2, w))
                    if p == 1 and pending_drain is not None:
                        drain(*pending_drain)
                        pending_drain = None
                dma_eng = nc.sync if jb % 2 == 0 else nc.scalar
                pending_drain = (jb, o, w, dma_eng)
            for q in pend_pv:
                pv(*q, last=(q[0] == npairs - 1))
            drain(*pending_drain)

    nc.compile()
    _build_cache[key] = nc
    return nc


def _run(inputs, trace=False):
    queries = np.asarray(inputs["queries"], dtype=np.float32)
    keys = np.asarray(inputs["keys"], dtype=np.float32)
    values = np.asarray(inputs["values"], dtype=np.float32)
    mask_query = np.asarray(inputs["mask_query"])
    mask_key = np.asarray(inputs["mask_key"])

    b, n, d = queries.shape
    dv = values.shape[2]
    assert b == N_CORES, f"batch {b} != {N_CORES} cores"
    bf = ml_dtypes.bfloat16

    idx_q = [np.flatnonzero(mask_query[i]) for i in range(b)]
    idx_k = [np.flatnonzero(mask_key[i]) for i in range(b)]
    ncap = max(max(len(ix) for ix in idx_q), 64)
    mcap = _round_up(max(max(len(ix) for ix in idx_k), 1), 128)
    mchunks = mcap // 128
    npairs = (mchunks + 1) // 2
    blocks = _blocks_of(ncap)
    nblocks = len(blocks)

    qt = np.zeros((b, 128, ncap), bf)
    ktp = np.zeros((b, 128, npairs * 128), bf)
    vext = np.zeros((b, 128, mchunks * 65), bf)
    for i in range(b):
        nq, nk = len(idx_q[i]), len(idx_k[i])
        qc_t = queries[i, idx_q[i]].T.astype(bf)
        qt[i, 0:64, :nq] = qc_t
        qt[i, 64:128, :nq] = qc_t
        kc_t = np.zeros((64, mcap), np.float32)
        kc_t[:, :nk] = keys[i, idx_k[i]].T
        kc_t = kc_t.astype(bf)
        for p in range(npairs):
            ktp[i, 0:64, p * 128:(p + 1) * 128] = \
                kc_t[:, (2 * p) * 128:(2 * p + 1) * 128]
            if 2 * p + 1 < mchunks:
                ktp[i, 64:128, p * 128:(p + 1) * 128] = \
                    kc_t[:, (2 * p + 1) * 128:(2 * p + 2) * 128]
        ve = np.zeros((mcap, 65), np.float32)
        ve[:nk, :dv] = values[i, idx_k[i]]
        ve[:nk, dv] = 1.0
        vext[i] = ve.reshape(mchunks, 128, 65).transpose(1, 0, 2) \
                    .reshape(128, mchunks * 65).astype(bf)

    nc = _build(ncap, mcap)

    from concourse.bass_utils import run_bass_kernel_spmd
    in_maps = [{"qt": qt[i], "ktp": ktp[i], "vext": vext[i]}
               for i in range(b)]
    res = run_bass_kernel_spmd(nc, in_maps, core_ids=list(range(N_CORES)),
                               trace=trace)

    out = np.empty((b, n, dv), np.float32)
    for i in range(b):
        ot = res.results[i]["out"]
        nq, nk = len(idx_q[i]), len(idx_k[i])
        full = np.concatenate(
            [ot[jb * 65:(jb + 1) * 65, :w] for jb, (j0, w)
             in enumerate(blocks)], axis=1)
        num = full[:dv, :nq]
        den = full[dv, :nq]
        if nk > 0:
            out[i, :, :] = values[i, idx_k[i]].mean(axis=0)
        else:
            out[i, :, :] = 0.0
        if nq > 0:
            out[i, idx_q[i], :] = (num / den).T
    return out, res


def kernel(**inputs):
    out, _ = _run(inputs, trace=False)
    return out
